# revision 1
# baseline (speedup 1.0000x reference)
"""Trainium2 Bass kernel for nn_Explainer: out[b] = sum_k w[b,k] * (archs[k] off-diag).

Equivalent to a (2048,32) @ (32,65536) fp32 matmul with the diagonal of each
256x256 archetype zeroed. Sharding: the 65536 output columns are split across
the 8 cores (8192 columns each) — each core then only needs a 1 MB slice of
the archetypes plus the full (small) weight matrix, minimizing HBM traffic;
the 64 MB/core output write is the roofline.

Per-core device layout (all host-side prepped so every DMA is a plain copy):
  wt4   (128, 2048): batch_weights^T replicated into 4 row-groups
                     wt4[32a+k, b] = w[b, k]
  archp (128, 2048): the core's 8192 archetype columns, masked, split into
                     16 chunks of 512; chunk t lives in row-group a = t%4 at
                     quad j = t//4: archp[32a+k, 512j+c] = A[k, 512t+c]
  out   (2048, 8192): the core's output column slice, natural order.

Compute: for each 128-row batch tile, 4 quads x 4 row-tiled fp32 matmuls
(K=32 at tile_position (32a,0) run concurrently on the PE), PSUM drained by
alternating VectorE/ScalarE copies into an SBUF staging tile, staged tiles
DMA'd out as single 4 MB contiguous writes.
"""

import numpy as np

import concourse.tile as tile
from concourse import bacc, mybir
from concourse.bass_utils import run_bass_kernel_spmd

B, K, D = 2048, 32, 256
NCORES = 8
COLS = D * D            # 65536
CPC = COLS // NCORES    # 8192 columns per core
GW = CPC // 4           # 2048 columns per row-group
MT = 128                # batch tile rows (psum partition dim)
NMT = B // MT           # 16 batch tiles
NQ = GW // 512          # 4 quads per batch tile

F32 = mybir.dt.float32

_compiled = {}


def _build():
    nc = bacc.Bacc("TRN2", target_bir_lowering=False, debug=False, num_devices=NCORES)
    wt = nc.dram_tensor("wt4", [128, B], F32, kind="ExternalInput").ap()
    ar = nc.dram_tensor("archp", [128, GW], F32, kind="ExternalInput").ap()
    out = nc.dram_tensor("out", [B, CPC], F32, kind="ExternalOutput").ap()

    with tile.TileContext(nc) as tc:
        with (
            tc.tile_pool(name="wpool", bufs=1) as wpool,
            tc.tile_pool(name="apool", bufs=1) as apool,
            tc.tile_pool(name="pspool", bufs=2, space="PSUM") as pspool,
            tc.tile_pool(name="stpool", bufs=3) as stpool,
        ):
            wt_sb = wpool.tile([128, B], F32)
            nc.sync.dma_start(wt_sb[:], wt[:])
            ar_sb = apool.tile([128, GW], F32)
            nc.sync.dma_start(ar_sb[:], ar[:])

            cnt = 0
            for m in range(NMT):
                st = stpool.tile([128, CPC], F32)
                for j in range(NQ):
                    ps = pspool.tile([128, 4 * 512], F32)
                    for a in range(4):
                        nc.tensor.matmul(
                            ps[:, 512 * a : 512 * (a + 1)],
                            wt_sb[32 * a : 32 * (a + 1), MT * m : MT * (m + 1)],
                            ar_sb[32 * a : 32 * (a + 1), 512 * j : 512 * (j + 1)],
                            start=True,
                            stop=True,
                            tile_position=(32 * a, 0),
                        )
                    if cnt % 2 == 0:
                        nc.vector.tensor_copy(st[:, GW * j : GW * (j + 1)], ps[:])
                    else:
                        nc.scalar.copy(st[:, GW * j : GW * (j + 1)], ps[:])
                    cnt += 1
                nc.sync.dma_start(out[MT * m : MT * (m + 1), :], st[:])

    nc.compile()
    return nc


def _get_nc():
    if "nc" not in _compiled:
        _compiled["nc"] = _build()
    return _compiled["nc"]


def _prep_inputs(batch_weights: np.ndarray, archs: np.ndarray):
    w = np.ascontiguousarray(np.asarray(batch_weights, dtype=np.float32))
    A = np.asarray(archs, dtype=np.float32).reshape(K, COLS).copy()
    A[:, :: D + 1] = 0.0  # zero the diagonal of each (D, D) archetype

    wt4 = np.ascontiguousarray(np.tile(w.T, (4, 1)))  # (128, B)

    in_maps = []
    for c in range(NCORES):
        sl = A[:, CPC * c : CPC * (c + 1)].reshape(K, 4 * NQ, 512)
        archp = np.concatenate(
            [sl[:, a::4, :].reshape(K, GW) for a in range(4)], axis=0
        )  # (128, GW)
        in_maps.append({"wt4": wt4, "archp": np.ascontiguousarray(archp)})
    return in_maps


def _gather(results) -> np.ndarray:
    outf = np.empty((B, COLS), dtype=np.float32)
    for c in range(NCORES):
        outf[:, CPC * c : CPC * (c + 1)] = results[c]["out"]
    return outf.reshape(B, D, D)


def kernel(batch_weights: np.ndarray, archs: np.ndarray, **run_kwargs) -> np.ndarray:
    nc = _get_nc()
    in_maps = _prep_inputs(batch_weights, archs)
    res = run_bass_kernel_spmd(nc, in_maps, list(range(NCORES)), **run_kwargs)
    if run_kwargs:
        _compiled["last_result"] = res
    return _gather(res.results)


# revision 2
# speedup vs baseline: 1.0404x; 1.0404x over previous
"""Trainium2 Bass kernel for nn_Explainer: out[b] = sum_k w[b,k] * (archs[k] off-diag).

Equivalent to a (2048,32) @ (32,65536) fp32 matmul with the diagonal of each
256x256 archetype zeroed. Sharding: the 65536 output columns are split across
the 8 cores (8192 columns each) — each core then only needs a 1 MB slice of
the archetypes plus the full (small) weight matrix, minimizing HBM traffic;
the 64 MB/core output write is the roofline.

Per-core device layout (all host-side prepped so every DMA is a plain copy):
  wt4   (128, 2048): batch_weights^T replicated into 4 row-groups
                     wt4[32a+k, b] = w[b, k]
  archp (128, 2048): the core's 8192 archetype columns, masked, split into
                     16 chunks of 512; chunk t lives in row-group a = t%4 at
                     quad j = t//4: archp[32a+k, 512j+c] = A[k, 512t+c]
  out   (2048, 8192): the core's output column slice, natural order.

Compute: for each 128-row batch tile, 4 quads x 4 row-tiled fp32 matmuls
(K=32 at tile_position (32a,0) run concurrently on the PE), PSUM drained by
alternating VectorE/ScalarE copies into an SBUF staging tile, staged tiles
DMA'd out as single 4 MB contiguous writes.
"""

import numpy as np

import concourse.tile as tile
from concourse import bacc, mybir
from concourse.bass_utils import run_bass_kernel_spmd

B, K, D = 2048, 32, 256
NCORES = 8
COLS = D * D            # 65536
CPC = COLS // NCORES    # 8192 columns per core
GW = CPC // 4           # 2048 columns per row-group
MT = 128                # batch tile rows (psum partition dim)
NMT = B // MT           # 16 batch tiles
NQ = GW // 512          # 4 quads per batch tile

F32 = mybir.dt.float32

_compiled = {}


def _build():
    nc = bacc.Bacc("TRN2", target_bir_lowering=False, debug=False, num_devices=NCORES)
    wt = nc.dram_tensor("wt4", [128, B], F32, kind="ExternalInput").ap()
    ar = nc.dram_tensor("archp", [128, GW], F32, kind="ExternalInput").ap()
    out = nc.dram_tensor("out", [B, CPC], F32, kind="ExternalOutput").ap()

    with tile.TileContext(nc) as tc:
        with (
            tc.tile_pool(name="wpool", bufs=1) as wpool,
            tc.tile_pool(name="apool", bufs=1) as apool,
            tc.tile_pool(name="pspool", bufs=2, space="PSUM") as pspool,
            tc.tile_pool(name="stpool", bufs=8) as stpool,
        ):
            # Chunked input loads so the first matmuls (needing only
            # wt4[:, :128] and archp[:, :512]) start ~2 us in rather than
            # waiting for the full 2 MB of inputs.
            wt_sb = wpool.tile([128, B], F32)
            ar_sb = apool.tile([128, GW], F32)
            nc.sync.dma_start(wt_sb[:, :MT], wt[:, :MT])
            for j in range(NQ):
                nc.sync.dma_start(
                    ar_sb[:, 512 * j : 512 * (j + 1)], ar[:, 512 * j : 512 * (j + 1)]
                )
            nc.sync.dma_start(wt_sb[:, MT:], wt[:, MT:])

            cnt = 0
            for m in range(NMT):
                for j in range(NQ):
                    ps = pspool.tile([128, 4 * 512], F32)
                    for a in range(4):
                        nc.tensor.matmul(
                            ps[:, 512 * a : 512 * (a + 1)],
                            wt_sb[32 * a : 32 * (a + 1), MT * m : MT * (m + 1)],
                            ar_sb[32 * a : 32 * (a + 1), 512 * j : 512 * (j + 1)],
                            start=True,
                            stop=True,
                            tile_position=(32 * a, 0),
                        )
                    st = stpool.tile([128, GW], F32)
                    if cnt % 2 == 0:
                        nc.vector.tensor_copy(st[:], ps[:])
                    else:
                        nc.scalar.copy(st[:], ps[:])
                    cnt += 1
                    nc.sync.dma_start(
                        out[MT * m : MT * (m + 1), GW * j : GW * (j + 1)], st[:]
                    )

    nc.compile()
    return nc


def _get_nc():
    if "nc" not in _compiled:
        _compiled["nc"] = _build()
    return _compiled["nc"]


def _prep_inputs(batch_weights: np.ndarray, archs: np.ndarray):
    w = np.ascontiguousarray(np.asarray(batch_weights, dtype=np.float32))
    A = np.asarray(archs, dtype=np.float32).reshape(K, COLS).copy()
    A[:, :: D + 1] = 0.0  # zero the diagonal of each (D, D) archetype

    wt4 = np.ascontiguousarray(np.tile(w.T, (4, 1)))  # (128, B)

    in_maps = []
    for c in range(NCORES):
        sl = A[:, CPC * c : CPC * (c + 1)].reshape(K, 4 * NQ, 512)
        archp = np.concatenate(
            [sl[:, a::4, :].reshape(K, GW) for a in range(4)], axis=0
        )  # (128, GW)
        in_maps.append({"wt4": wt4, "archp": np.ascontiguousarray(archp)})
    return in_maps


def _gather(results) -> np.ndarray:
    outf = np.empty((B, COLS), dtype=np.float32)
    for c in range(NCORES):
        outf[:, CPC * c : CPC * (c + 1)] = results[c]["out"]
    return outf.reshape(B, D, D)


def kernel(batch_weights: np.ndarray, archs: np.ndarray, **run_kwargs) -> np.ndarray:
    nc = _get_nc()
    in_maps = _prep_inputs(batch_weights, archs)
    res = run_bass_kernel_spmd(nc, in_maps, list(range(NCORES)), **run_kwargs)
    if run_kwargs:
        _compiled["last_result"] = res
    return _gather(res.results)
